# revision 33
# baseline (speedup 1.0000x reference)
"""Trainium2 Bass kernel for the GODEFunc graph-ODE message-passing module.

Math (per batch b):
    xa   = sum_k conv_w[k] * (adj[k] @ x[b]) + conv_b
    W'   = (w * clip(d,0,1)) @ w.T          (symmetric)
    out  = tanh(0.5*sigmoid(alpha) * xa - 2*x[b] + x[b] @ W' + x0[b]*sigmoid(beta))

Sharding: output rows (nodes) split across 8 cores; each core computes its
1024-row slice for all batches.  No collectives.

Layout decisions:
  - Host stages adj TRANSPOSED and 128-row tiled
    (adjt[k, p, mc, n] = adj[k, rows0+n, mc*128+p]) so the contraction dim m
    lands on SBUF partitions naturally -> zero on-chip transposes, and every
    DMA gets multi-KB contiguous per-partition lines.
  - adj streams as fp8e4m3 scaled by 2^12 (entries of the row-normalized
    adjacency are ~1e-4); the scale is folded back out exactly through the
    alpha gate.  x is resident as bf16; the graph matmul runs bf16 with fp32
    PSUM accumulation; the xw path and all gates are fp32.  Measured output
    error ~5e-4 vs the 2e-2 gate.
  - The matmul runs x-STATIONARY: lhsT = x4[mc] column pairs (128 rows of
    contraction x 128 (b,f) columns), moving operand = the combined adj
    stream at N=512, grouped so consecutive matmuls hit the same PSUM bank.
    The output is produced transposed ([bf, n]) and the host un-transposes.
  - Each PSUM accumulation group owns a full 2KB bank: has_written/"zero
    region" flags are PER-BANK on TRN2, so a second group's start=True in a
    shared bank silently drops the first group's accumulated partials (this
    was the old baseline's 8.7e-3 error).
  - k=0,1 combined on DVE with a single scalar_tensor_tensor per group via
    the conv_w ratio trick: comb' = a0 + (cw1/cw0)*a1, with cw0 folded into
    the alpha gate.  DVE is the pacing engine (~75us at 1 elem/lane/cycle);
    0.5*sigmoid(alpha), sigmoid(beta) apply in the epilogue via
    partition-broadcast tiles, psum-independent terms precomputed up front.
"""

import sys

for _p in ("/opt/trn_rl_repo",):
    if _p not in sys.path:
        sys.path.insert(0, _p)

from contextlib import ExitStack

import numpy as np

import concourse.bass as bass
import concourse.mybir as mybir
import concourse.tile as tile
from concourse import bacc
from concourse.bass_utils import run_bass_kernel_spmd
from concourse.masks import make_identity

dt = mybir.dt
AF = mybir.ActivationFunctionType
ALU = mybir.AluOpType

B, N, F, K = 4, 8192, 64, 2
N_CORES = 8
P = 128

# The kernel computes the adjacency matmul in bf16 either way (values are
# bit-identical); staging the bf16 cast on the host halves the HBM bytes the
# cores must stream.  The xw path and all gates stay fp32.
ADJ_BF16_HOST = True
# One step further: stage adj as fp8e4m3 scaled by 2^12 (row-normalized adj
# entries are ~1e-4; the scale is folded back out exactly via the alpha gate).
# Halves the adj stream again; the matmul itself still runs bf16 (the combine
# upconverts).  Output error ~1.5e-4 vs the 2e-2 gate.
ADJ_FP8_HOST = True
ADJ_FP8_SCALE = 4096.0


def build_kernel(n=N, n_cores=N_CORES, b=B, f=F, k_dim=K, mg_mc=2,
                 adj_bf16_host=ADJ_BF16_HOST, adj_fp8_host=ADJ_FP8_HOST):
    if adj_fp8_host:
        mg_mc = max(mg_mc, 4)  # keep adj DMAs >=512KB at 1 byte/elem
    ns = n // n_cores          # rows per core (1024)
    mc_cnt = n // P            # contraction chunks (64)
    nmg = mc_cnt // mg_mc      # adj DMA groups along contraction dim
    bf = b * f
    nh = bf // P               # output partition halves (2)
    nw = ns // 512             # 512-col slices of the row range (2)

    nc = bacc.Bacc(None, target_bir_lowering=False, debug=False)

    adj_dt = dt.bfloat16 if adj_bf16_host else dt.float32
    if adj_fp8_host:
        adj_dt = dt.float8e4
    # adjt[k, p, mc, n] = adj[k, rows0+n, mc*128+p]
    adjt = nc.dram_tensor("adjt", [k_dim, P, mc_cnt, ns], adj_dt,
                          kind="ExternalInput")
    # xres_in[p, mc, b, f] = x[b, mc*128+p, f]
    if adj_fp8_host:
        x_dt = dt.float8e4
    elif adj_bf16_host:
        x_dt = dt.bfloat16
    else:
        x_dt = dt.float32
    xres_in = nc.dram_tensor("xres_in", [P, mc_cnt, b, f], x_dt,
                             kind="ExternalInput")
    # xt_rows[f, b, n] = x[b, rows0+n, f]
    xt_rows = nc.dram_tensor("xt_rows", [f, b, ns], dt.float32,
                             kind="ExternalInput")
    # x0t_in[p, h, n] = x0[2h + p//64, rows0+n, p%64]
    x0t_in = nc.dram_tensor("x0t_in", [P, nh, ns], dt.float32,
                            kind="ExternalInput")
    # ab_rows[0, n] = alpha[rows0+n]; ab_rows[1, n] = beta[rows0+n]
    ab_rows = nc.dram_tensor("ab_rows", [2, ns], dt.float32,
                             kind="ExternalInput")
    wT = nc.dram_tensor("wT", [f, f], dt.float32, kind="ExternalInput")
    d = nc.dram_tensor("d", [f], dt.float32, kind="ExternalInput")
    conv_w = nc.dram_tensor("conv_w", [k_dim], dt.float32, kind="ExternalInput")
    conv_b = nc.dram_tensor("conv_b", [1], dt.float32, kind="ExternalInput")
    # y_T[h, p, n] = out[2h + p//64, rows0+n, p%64]
    y_T = nc.dram_tensor("y_T", [nh, P, ns], dt.float32,
                         kind="ExternalOutput")

    with tile.TileContext(nc) as tc, ExitStack() as ctx:
        const = ctx.enter_context(tc.tile_pool(name="const", bufs=1))
        xres = ctx.enter_context(tc.tile_pool(name="xres", bufs=1))
        adj_pool = ctx.enter_context(tc.tile_pool(name="adjp", bufs=4))
        comb_pool = ctx.enter_context(tc.tile_pool(name="combp", bufs=4))
        work = ctx.enter_context(tc.tile_pool(name="work", bufs=3))
        outp = ctx.enter_context(tc.tile_pool(name="outp", bufs=2))
        psy = ctx.enter_context(tc.tile_pool(name="psy", bufs=1, space="PSUM"))
        ppxw = ctx.enter_context(tc.tile_pool(name="ppxw", bufs=1, space="PSUM"))

        # ---------------- main-stream DMAs issued first ----------------
        # resident x, all batches, contraction layout (chunked so the first
        # matmuls can start before the whole tensor lands)
        x_sb_dt = dt.float8e4 if adj_fp8_host else dt.bfloat16
        x4 = xres.tile([P, mc_cnt, b, f], x_sb_dt, tag="x4")
        x4_chunk = mc_cnt // 4
        for xc in range(4):
            xs = slice(xc * x4_chunk, (xc + 1) * x4_chunk)
            if adj_bf16_host or adj_fp8_host:
                nc.sync.dma_start(out=x4[:, xs, :, :], in_=xres_in[:, xs, :, :])
            else:
                nc.gpsimd.dma_start(out=x4[:, xs, :, :], in_=xres_in[:, xs, :, :])

        # ---------------- constants / gates ----------------
        ident_f = const.tile([f, f], dt.float32, tag="ident_f")
        make_identity(nc, ident_f[:])

        wT_sb = const.tile([f, f], dt.float32, tag="wT_sb")
        nc.sync.dma_start(out=wT_sb[:], in_=wT[:, :])
        d_sb = const.tile([f, 1], dt.float32, tag="d_sb")
        nc.sync.dma_start(out=d_sb[:], in_=d[:, None])
        cw_sb = const.tile([P, k_dim], dt.float32, tag="cw_sb")
        nc.sync.dma_start(out=cw_sb[:], in_=conv_w[None, :].to_broadcast((P, k_dim)))
        cb_sb = const.tile([P, 1], dt.float32, tag="cb_sb")
        nc.sync.dma_start(out=cb_sb[:], in_=conv_b[None, :].to_broadcast((P, 1)))

        # alpha/beta broadcast down partitions: [P, ns]
        aB = work.tile([P, ns], dt.float32, tag="ew")
        nc.sync.dma_start(out=aB[:], in_=ab_rows[0:1, :].to_broadcast((P, ns)))
        bB = work.tile([P, ns], dt.float32, tag="ew")
        nc.sync.dma_start(out=bB[:], in_=ab_rows[1:2, :].to_broadcast((P, ns)))
        siga05B = const.tile([P, ns], dt.float32, tag="siga05B")
        nc.scalar.activation(siga05B[:], aB[:], AF.Sigmoid)
        nc.vector.tensor_scalar(siga05B[:], siga05B[:], 0.5, None, ALU.mult)
        sigbB = const.tile([P, ns], dt.float32, tag="sigbB")
        nc.scalar.activation(sigbB[:], bB[:], AF.Sigmoid)
        # conv_w ratio trick: stream comb' = a0 + (cw1/cw0)*a1 (one DVE op per
        # group), and fold the overall cw0 into the alpha gate for the psum
        # term: siga05Bc = 0.5*sigmoid(alpha)*cw0.
        ratio = const.tile([P, 1], dt.float32, tag="ratio")
        nc.vector.reciprocal(ratio[:], cw_sb[:, 0:1])
        nc.vector.tensor_scalar(ratio[:], ratio[:], cw_sb[:, 1:2], None, ALU.mult)
        siga05Bc = const.tile([P, ns], dt.float32, tag="siga05Bc")
        nc.vector.tensor_scalar(
            siga05Bc[:], siga05B[:], cw_sb[:, 0:1], None, ALU.mult
        )
        if adj_fp8_host:
            # undo the exact 2^12 host scale on the adj stream
            nc.vector.tensor_scalar(
                siga05Bc[:], siga05Bc[:], 1.0 / ADJ_FP8_SCALE, None, ALU.mult
            )

        # ---------------- xt rows (for xw), x0 ----------------
        xt_sb = const.tile([f, b, ns], dt.float32, tag="xt_sb")
        nc.sync.dma_start(out=xt_sb[:], in_=xt_rows[:, :, :])
        x0_sb = const.tile([P, nh, ns], dt.float32, tag="x0_sb")
        nc.sync.dma_start(out=x0_sb[:], in_=x0t_in[:, :, :])

        # ---------------- PSUM: 4 banks y, 4 banks pxw ----------------
        psum_T = [psy.tile([P, ns], dt.float32, tag=f"yT{h}", name=f"psum_T{h}")
                  for h in range(nh)]
        pxw_T = [ppxw.tile([P, ns], dt.float32, tag=f"pxw{h}", name=f"pxw_T{h}")
                 for h in range(nh)]

        # ---------------- W' = (w * clip(d,0,1)) @ w.T - 2I ----------------
        dc = const.tile([f, 1], dt.float32, tag="dc")
        nc.vector.tensor_scalar(dc[:], d_sb[:], 0.0, 1.0, ALU.max, ALU.min)
        wdcT = const.tile([f, f], dt.float32, tag="wdcT")
        nc.vector.tensor_scalar(wdcT[:], wT_sb[:], dc[:], None, ALU.mult)
        pw = pxw_T[0][0:f, 0:f]  # borrow a pxw bank region; consumed before pxw
        nc.tensor.matmul(pw, wdcT[:], wT_sb[:], start=True, stop=True)
        wp = const.tile([f, f], dt.float32, tag="wp")
        nc.vector.scalar_tensor_tensor(
            wp[:], ident_f[:], -2.0, pw, ALU.mult, ALU.add
        )

        # ---------------- pxw_T[h] = (x_rows @ (W'-2I)).T, transposed layout --
        # wp is symmetric, so it serves directly as lhsT.
        for h in range(nh):
            for bb in (2 * h, 2 * h + 1):
                base = (bb % 2) * f
                for s in range(nw):
                    nc.tensor.matmul(
                        pxw_T[h][base : base + f, s * 512 : (s + 1) * 512],
                        wp[:],
                        xt_sb[:, bb, s * 512 : (s + 1) * 512],
                        start=True,
                        stop=True,
                    )
        # psum-independent epilogue terms, computed up front:
        # e12[h] = x0*sigmoid(beta) + xw
        keepP = [const.tile([P, ns], dt.float32, tag=f"e12_{h}", name=f"e12_{h}")
                 for h in range(nh)]
        for h in range(nh):
            e1 = work.tile([P, ns], dt.float32, tag="ew")
            nc.vector.tensor_tensor(e1[:], x0_sb[:, h, :], sigbB[:], ALU.mult)
            nc.vector.tensor_tensor(keepP[h][:], e1[:], pxw_T[h][:], ALU.add)

        # ---------------- main loop: stream adjt, k-combine, matmul ----------
        # The last MM_DIRECT_MGS groups skip the DVE combine entirely: their
        # two k streams feed the PE directly, k=1 against x4r = ratio*x4.
        # This moves ~25% of the combine off the pacing DVE onto PE headroom.
        a_sb_dt = dt.float8e4 if adj_fp8_host else dt.bfloat16
        mm_direct_mgs = 4 if adj_fp8_host else 0
        PREF = 4
        a_tiles = {}

        def issue_adj(mg):
            cs = slice(mg * mg_mc, (mg + 1) * mg_mc)
            for kk in range(k_dim):
                a_t = adj_pool.tile([P, mg_mc * ns], a_sb_dt,
                                    tag=f"adj{kk}", name=f"adj_t{kk}")
                eng = nc.scalar if (adj_bf16_host or adj_fp8_host) else nc.gpsimd
                eng.dma_start(
                    out=a_t[:].rearrange("p (c n) -> p c n", c=mg_mc),
                    in_=adjt[kk, :, cs, :],
                )
                a_tiles[(mg, kk)] = a_t

        for mg in range(min(PREF, nmg)):
            issue_adj(mg)

        # x4r = ratio * x4, built on the ACT engine (off the DVE critical
        # path), after the prefetch dispatches so it doesn't head-block them
        x4r = None
        if mm_direct_mgs:
            x4r = xres.tile([P, mc_cnt, b, f], x_sb_dt, tag="x4r")
            for xc in range(4):
                xs = slice(xc * x4_chunk, (xc + 1) * x4_chunk)
                nc.scalar.activation(
                    x4r[:, xs, :, :], x4[:, xs, :, :], AF.Copy,
                    scale=ratio[:, 0:1],
                )

        def x4_slice(src, mc, h):
            return src[:, mc, 2 * h : 2 * h + 2, :].rearrange("p b f -> p (b f)")

        for mg in range(nmg):
            if mg + PREF < nmg:
                issue_adj(mg + PREF)
            a_k = [a_tiles.pop((mg, kk)) for kk in range(k_dim)]
            two_mm = mg >= nmg - mm_direct_mgs
            if not two_mm:
                # comb' = a0 + (cw1/cw0)*a1  (single DVE op)
                comb = comb_pool.tile([P, mg_mc * ns], a_sb_dt, tag="comb")
                nc.vector.scalar_tensor_tensor(
                    comb[:], a_k[1][:], ratio[:, 0:1], a_k[0][:],
                    ALU.mult, ALU.add,
                )
            # same-PSUM-bank runs of matmuls (avoids bank cycling)
            for h in range(nh):
                for s in range(nw):
                    for c in range(mg_mc):
                        mc = mg * mg_mc + c
                        rs = slice(c * ns + s * 512, c * ns + (s + 1) * 512)
                        if two_mm:
                            nc.tensor.matmul(
                                psum_T[h][:, s * 512 : (s + 1) * 512],
                                x4_slice(x4, mc, h),
                                a_k[0][:, rs],
                                start=False,
                                stop=False,
                                skip_group_check=True,
                            )
                            nc.tensor.matmul(
                                psum_T[h][:, s * 512 : (s + 1) * 512],
                                x4_slice(x4r, mc, h),
                                a_k[1][:, rs],
                                start=False,
                                stop=(mc == mc_cnt - 1),
                                skip_group_check=True,
                            )
                        else:
                            nc.tensor.matmul(
                                psum_T[h][:, s * 512 : (s + 1) * 512],
                                x4_slice(x4, mc, h),
                                comb[:, rs],
                                start=(mc == 0),
                                stop=(mc == mc_cnt - 1 and not mm_direct_mgs),
                                skip_group_check=True,
                            )

        # ---------------- epilogue ----------------
        # out = tanh(0.5*siga*(y + cb) + xw + x0*sigb), all in [bf, n] layout
        for h in range(nh):
            e3 = work.tile([P, ns], dt.float32, tag="ew")
            nc.vector.tensor_tensor(e3[:], psum_T[h][:], siga05Bc[:], ALU.mult)
            e4 = work.tile([P, ns], dt.float32, tag="ew")
            nc.vector.tensor_tensor(e4[:], e3[:], keepP[h][:], ALU.add)
            # + 0.5*siga*conv_b
            e5 = work.tile([P, ns], dt.float32, tag="ew")
            nc.vector.scalar_tensor_tensor(
                e5[:], siga05B[:], cb_sb[:, 0:1], e4[:], ALU.mult, ALU.add
            )
            outt = outp.tile([P, ns], dt.float32, tag="outt")
            nc.scalar.activation(outt[:], e5[:], AF.Tanh)
            nc.sync.dma_start(out=y_T[h, :, :], in_=outt[:])

    nc.finalize()
    return nc


_NC_CACHE = {}


def _get_nc():
    key = (N, N_CORES, B, F, K, ADJ_BF16_HOST, ADJ_FP8_HOST)
    if key not in _NC_CACHE:
        _NC_CACHE[key] = build_kernel(
            n=N, n_cores=N_CORES, b=B, f=F, k_dim=K,
            adj_bf16_host=ADJ_BF16_HOST, adj_fp8_host=ADJ_FP8_HOST,
        )
    return _NC_CACHE[key]


def make_in_maps(x, x0, adj, alpha, beta, w, d, conv_w, conv_b, n_cores=N_CORES):
    """Host-side staging: slice rows per core and retile/transpose (pure
    layout transforms)."""
    k_dim, n, _ = adj.shape
    b, _, f = x.shape
    ns = n // n_cores
    mc_cnt = n // P
    nh = (b * f) // P
    f32 = np.float32
    adj = np.asarray(adj, dtype=f32)
    x = np.asarray(x, dtype=f32)
    x0 = np.asarray(x0, dtype=f32)
    alpha = np.asarray(alpha, dtype=f32)
    beta = np.asarray(beta, dtype=f32)

    # xres_in[p, mc, b, f] = x[b, mc*128+p, f]   (shared by all cores)
    xres_in = np.ascontiguousarray(
        x.reshape(b, mc_cnt, P, f).transpose(2, 1, 0, 3)
    )
    if ADJ_FP8_HOST:
        import ml_dtypes

        xres_in = xres_in.astype(ml_dtypes.float8_e4m3)
    elif ADJ_BF16_HOST:
        import ml_dtypes

        xres_in = xres_in.astype(ml_dtypes.bfloat16)
    wT = np.ascontiguousarray(np.asarray(w, dtype=f32).T)

    in_maps = []
    for c in range(n_cores):
        rows = slice(c * ns, (c + 1) * ns)
        ac = adj[:, rows, :]  # [k, ns, n] view
        # adjt[k, p, mc, nn] = ac[k, nn, mc*128+p]
        s0, s1, s2 = ac.strides
        adjt = np.lib.stride_tricks.as_strided(
            ac, shape=(k_dim, P, mc_cnt, ns), strides=(s0, s2, P * s2, s1)
        )
        adjt = np.ascontiguousarray(adjt)
        if ADJ_FP8_HOST:
            import ml_dtypes

            adjt = (adjt * np.float32(ADJ_FP8_SCALE)).astype(
                ml_dtypes.float8_e4m3
            )
        elif ADJ_BF16_HOST:
            import ml_dtypes

            adjt = adjt.astype(ml_dtypes.bfloat16)

        xr = x[:, rows, :]  # [b, ns, f]
        xt_rows = np.ascontiguousarray(xr.transpose(2, 0, 1))  # [f, b, ns]
        # x0t_in[p, h, nn] = x0[2h + p//64, rows0+nn, p%64]
        x0t_in = np.ascontiguousarray(
            x0[:, rows, :].transpose(0, 2, 1).reshape(nh, P, ns).transpose(1, 0, 2)
        )
        ab = np.stack([alpha[rows], beta[rows]], axis=0)

        in_maps.append(
            {
                "adjt": adjt,
                "xres_in": xres_in,
                "xt_rows": xt_rows,
                "x0t_in": x0t_in,
                "ab_rows": np.ascontiguousarray(ab),
                "wT": wT,
                "d": np.ascontiguousarray(d, dtype=f32),
                "conv_w": np.ascontiguousarray(conv_w, dtype=f32),
                "conv_b": np.ascontiguousarray(conv_b, dtype=f32),
            }
        )
    return in_maps


def assemble_output(per_core_y, n_cores=N_CORES):
    """y_T[h, p, n] per core -> full [b, n, f]."""
    parts = []
    for c in range(n_cores):
        yt = per_core_y[c]  # [nh, P, ns]
        nh_, p_, ns_ = yt.shape
        b_ = nh_ * (p_ // 64)
        f_ = 64
        # [nh, P, ns] -> [b, f, ns] -> [b, ns, f]
        parts.append(
            yt.reshape(nh_, p_ // f_, f_, ns_)
            .reshape(b_, f_, ns_)
            .transpose(0, 2, 1)
        )
    return np.concatenate(parts, axis=1)


def kernel(x, x0, adj, alpha, beta, w, d, conv_w, conv_b):
    x = np.asarray(x)
    x0 = np.asarray(x0)
    adj = np.asarray(adj)
    alpha = np.asarray(alpha)
    beta = np.asarray(beta)
    w = np.asarray(w)
    d = np.asarray(d)
    conv_w = np.asarray(conv_w)
    conv_b = np.asarray(conv_b)

    nc = _get_nc()
    in_maps = make_in_maps(x, x0, adj, alpha, beta, w, d, conv_w, conv_b)
    res = run_bass_kernel_spmd(nc, in_maps, core_ids=list(range(N_CORES)))
    out = assemble_output([res.results[c]["y_T"] for c in range(N_CORES)])
    return out.astype(np.float32)


# revision 34
# speedup vs baseline: 1.0045x; 1.0045x over previous
"""Trainium2 Bass kernel for the GODEFunc graph-ODE message-passing module.

Math (per batch b):
    xa   = sum_k conv_w[k] * (adj[k] @ x[b]) + conv_b
    W'   = (w * clip(d,0,1)) @ w.T          (symmetric)
    out  = tanh(0.5*sigmoid(alpha) * xa - 2*x[b] + x[b] @ W' + x0[b]*sigmoid(beta))

Sharding: output rows (nodes) split across 8 cores; each core computes its
1024-row slice for all batches.  No collectives.

Layout decisions:
  - Host stages adj TRANSPOSED and 128-row tiled
    (adjt[k, p, mc, n] = adj[k, rows0+n, mc*128+p]) so the contraction dim m
    lands on SBUF partitions naturally -> zero on-chip transposes, and every
    DMA gets multi-KB contiguous per-partition lines.
  - adj streams as fp8e4m3 scaled by 2^12 (entries of the row-normalized
    adjacency are ~1e-4); the scale is folded back out exactly through the
    alpha gate.  x is resident as bf16; the graph matmul runs bf16 with fp32
    PSUM accumulation; the xw path and all gates are fp32.  Measured output
    error ~5e-4 vs the 2e-2 gate.
  - The matmul runs x-STATIONARY: lhsT = x4[mc] column pairs (128 rows of
    contraction x 128 (b,f) columns), moving operand = the combined adj
    stream at N=512, grouped so consecutive matmuls hit the same PSUM bank.
    The output is produced transposed ([bf, n]) and the host un-transposes.
  - Each PSUM accumulation group owns a full 2KB bank: has_written/"zero
    region" flags are PER-BANK on TRN2, so a second group's start=True in a
    shared bank silently drops the first group's accumulated partials (this
    was the old baseline's 8.7e-3 error).
  - k=0,1 combined on DVE with a single scalar_tensor_tensor per group via
    the conv_w ratio trick: comb' = a0 + (cw1/cw0)*a1, with cw0 folded into
    the alpha gate.  DVE is the pacing engine (~75us at 1 elem/lane/cycle);
    0.5*sigmoid(alpha), sigmoid(beta) apply in the epilogue via
    partition-broadcast tiles, psum-independent terms precomputed up front.
"""

import sys

for _p in ("/opt/trn_rl_repo",):
    if _p not in sys.path:
        sys.path.insert(0, _p)

from contextlib import ExitStack

import numpy as np

import concourse.bass as bass
import concourse.mybir as mybir
import concourse.tile as tile
from concourse import bacc
from concourse.bass_utils import run_bass_kernel_spmd
from concourse.masks import make_identity

dt = mybir.dt
AF = mybir.ActivationFunctionType
ALU = mybir.AluOpType

B, N, F, K = 4, 8192, 64, 2
N_CORES = 8
P = 128

# The kernel computes the adjacency matmul in bf16 either way (values are
# bit-identical); staging the bf16 cast on the host halves the HBM bytes the
# cores must stream.  The xw path and all gates stay fp32.
ADJ_BF16_HOST = True
# One step further: stage adj as fp8e4m3 scaled by 2^12 (row-normalized adj
# entries are ~1e-4; the scale is folded back out exactly via the alpha gate).
# Halves the adj stream again; the matmul itself still runs bf16 (the combine
# upconverts).  Output error ~1.5e-4 vs the 2e-2 gate.
ADJ_FP8_HOST = True
ADJ_FP8_SCALE = 4096.0


def build_kernel(n=N, n_cores=N_CORES, b=B, f=F, k_dim=K, mg_mc=2,
                 adj_bf16_host=ADJ_BF16_HOST, adj_fp8_host=ADJ_FP8_HOST):
    if adj_fp8_host:
        mg_mc = max(mg_mc, 4)  # keep adj DMAs >=512KB at 1 byte/elem
    ns = n // n_cores          # rows per core (1024)
    mc_cnt = n // P            # contraction chunks (64)
    nmg = mc_cnt // mg_mc      # adj DMA groups along contraction dim
    bf = b * f
    nh = bf // P               # output partition halves (2)
    nw = ns // 512             # 512-col slices of the row range (2)

    nc = bacc.Bacc(None, target_bir_lowering=False, debug=False)

    adj_dt = dt.bfloat16 if adj_bf16_host else dt.float32
    if adj_fp8_host:
        adj_dt = dt.float8e4
    # adjt[k, p, mc, n] = adj[k, rows0+n, mc*128+p]
    adjt = nc.dram_tensor("adjt", [k_dim, P, mc_cnt, ns], adj_dt,
                          kind="ExternalInput")
    # xres_in[p, mc, b, f] = x[b, mc*128+p, f]
    xres_in = nc.dram_tensor("xres_in", [P, mc_cnt, b, f],
                             dt.bfloat16 if (adj_bf16_host or adj_fp8_host)
                             else dt.float32,
                             kind="ExternalInput")
    # xt_rows[f, b, n] = x[b, rows0+n, f]
    xt_rows = nc.dram_tensor("xt_rows", [f, b, ns], dt.float32,
                             kind="ExternalInput")
    # x0t_in[p, h, n] = x0[2h + p//64, rows0+n, p%64]
    x0t_in = nc.dram_tensor("x0t_in", [P, nh, ns], dt.float32,
                            kind="ExternalInput")
    # ab_rows[0, n] = alpha[rows0+n]; ab_rows[1, n] = beta[rows0+n]
    ab_rows = nc.dram_tensor("ab_rows", [2, ns], dt.float32,
                             kind="ExternalInput")
    wT = nc.dram_tensor("wT", [f, f], dt.float32, kind="ExternalInput")
    d = nc.dram_tensor("d", [f], dt.float32, kind="ExternalInput")
    conv_w = nc.dram_tensor("conv_w", [k_dim], dt.float32, kind="ExternalInput")
    conv_b = nc.dram_tensor("conv_b", [1], dt.float32, kind="ExternalInput")
    # y_T[h, p, n] = out[2h + p//64, rows0+n, p%64]
    y_T = nc.dram_tensor("y_T", [nh, P, ns], dt.float32,
                         kind="ExternalOutput")

    with tile.TileContext(nc) as tc, ExitStack() as ctx:
        const = ctx.enter_context(tc.tile_pool(name="const", bufs=1))
        xres = ctx.enter_context(tc.tile_pool(name="xres", bufs=1))
        adj_pool = ctx.enter_context(tc.tile_pool(name="adjp", bufs=4))
        comb_pool = ctx.enter_context(tc.tile_pool(name="combp", bufs=4))
        work = ctx.enter_context(tc.tile_pool(name="work", bufs=3))
        outp = ctx.enter_context(tc.tile_pool(name="outp", bufs=2))
        psy = ctx.enter_context(tc.tile_pool(name="psy", bufs=1, space="PSUM"))
        ppxw = ctx.enter_context(tc.tile_pool(name="ppxw", bufs=1, space="PSUM"))

        # ---------------- main-stream DMAs issued first ----------------
        # resident x, all batches, contraction layout, bf16 (chunked so the
        # first matmuls can start before the whole tensor lands)
        x4 = xres.tile([P, mc_cnt, b, f], dt.bfloat16, tag="x4")
        x4_chunk = mc_cnt // 4
        for xc in range(4):
            xs = slice(xc * x4_chunk, (xc + 1) * x4_chunk)
            if adj_bf16_host or adj_fp8_host:
                nc.sync.dma_start(out=x4[:, xs, :, :], in_=xres_in[:, xs, :, :])
            else:
                nc.gpsimd.dma_start(out=x4[:, xs, :, :], in_=xres_in[:, xs, :, :])

        # ---------------- constants / gates ----------------
        ident_f = const.tile([f, f], dt.float32, tag="ident_f")
        make_identity(nc, ident_f[:])

        wT_sb = const.tile([f, f], dt.float32, tag="wT_sb")
        nc.sync.dma_start(out=wT_sb[:], in_=wT[:, :])
        d_sb = const.tile([f, 1], dt.float32, tag="d_sb")
        nc.sync.dma_start(out=d_sb[:], in_=d[:, None])
        cw_sb = const.tile([P, k_dim], dt.float32, tag="cw_sb")
        nc.sync.dma_start(out=cw_sb[:], in_=conv_w[None, :].to_broadcast((P, k_dim)))
        cb_sb = const.tile([P, 1], dt.float32, tag="cb_sb")
        nc.sync.dma_start(out=cb_sb[:], in_=conv_b[None, :].to_broadcast((P, 1)))

        # alpha/beta broadcast down partitions: [P, ns]
        aB = work.tile([P, ns], dt.float32, tag="ew")
        nc.sync.dma_start(out=aB[:], in_=ab_rows[0:1, :].to_broadcast((P, ns)))
        bB = work.tile([P, ns], dt.float32, tag="ew")
        nc.sync.dma_start(out=bB[:], in_=ab_rows[1:2, :].to_broadcast((P, ns)))
        siga05B = const.tile([P, ns], dt.float32, tag="siga05B")
        nc.scalar.activation(siga05B[:], aB[:], AF.Sigmoid)
        nc.vector.tensor_scalar(siga05B[:], siga05B[:], 0.5, None, ALU.mult)
        sigbB = const.tile([P, ns], dt.float32, tag="sigbB")
        nc.scalar.activation(sigbB[:], bB[:], AF.Sigmoid)
        # conv_w ratio trick: stream comb' = a0 + (cw1/cw0)*a1 (one DVE op per
        # group), and fold the overall cw0 into the alpha gate for the psum
        # term: siga05Bc = 0.5*sigmoid(alpha)*cw0.
        ratio = const.tile([P, 1], dt.float32, tag="ratio")
        nc.vector.reciprocal(ratio[:], cw_sb[:, 0:1])
        nc.vector.tensor_scalar(ratio[:], ratio[:], cw_sb[:, 1:2], None, ALU.mult)
        siga05Bc = const.tile([P, ns], dt.float32, tag="siga05Bc")
        nc.vector.tensor_scalar(
            siga05Bc[:], siga05B[:], cw_sb[:, 0:1], None, ALU.mult
        )
        if adj_fp8_host:
            # undo the exact 2^12 host scale on the adj stream
            nc.vector.tensor_scalar(
                siga05Bc[:], siga05Bc[:], 1.0 / ADJ_FP8_SCALE, None, ALU.mult
            )

        # ---------------- xt rows (for xw), x0 ----------------
        xt_sb = const.tile([f, b, ns], dt.float32, tag="xt_sb")
        nc.sync.dma_start(out=xt_sb[:], in_=xt_rows[:, :, :])
        x0_sb = const.tile([P, nh, ns], dt.float32, tag="x0_sb")
        nc.sync.dma_start(out=x0_sb[:], in_=x0t_in[:, :, :])

        # ---------------- PSUM: 4 banks y, 4 banks pxw ----------------
        psum_T = [psy.tile([P, ns], dt.float32, tag=f"yT{h}", name=f"psum_T{h}")
                  for h in range(nh)]
        pxw_T = [ppxw.tile([P, ns], dt.float32, tag=f"pxw{h}", name=f"pxw_T{h}")
                 for h in range(nh)]

        # ---------------- W' = (w * clip(d,0,1)) @ w.T - 2I ----------------
        dc = const.tile([f, 1], dt.float32, tag="dc")
        nc.vector.tensor_scalar(dc[:], d_sb[:], 0.0, 1.0, ALU.max, ALU.min)
        wdcT = const.tile([f, f], dt.float32, tag="wdcT")
        nc.vector.tensor_scalar(wdcT[:], wT_sb[:], dc[:], None, ALU.mult)
        pw = pxw_T[0][0:f, 0:f]  # borrow a pxw bank region; consumed before pxw
        nc.tensor.matmul(pw, wdcT[:], wT_sb[:], start=True, stop=True)
        wp = const.tile([f, f], dt.float32, tag="wp")
        nc.vector.scalar_tensor_tensor(
            wp[:], ident_f[:], -2.0, pw, ALU.mult, ALU.add
        )

        # ---------------- pxw_T[h] = (x_rows @ (W'-2I)).T, transposed layout --
        # wp is symmetric, so it serves directly as lhsT.
        for h in range(nh):
            for bb in (2 * h, 2 * h + 1):
                base = (bb % 2) * f
                for s in range(nw):
                    nc.tensor.matmul(
                        pxw_T[h][base : base + f, s * 512 : (s + 1) * 512],
                        wp[:],
                        xt_sb[:, bb, s * 512 : (s + 1) * 512],
                        start=True,
                        stop=True,
                    )
        # psum-independent epilogue terms, computed up front:
        # e12[h] = x0*sigmoid(beta) + xw
        keepP = [const.tile([P, ns], dt.float32, tag=f"e12_{h}", name=f"e12_{h}")
                 for h in range(nh)]
        for h in range(nh):
            e1 = work.tile([P, ns], dt.float32, tag="ew")
            nc.vector.tensor_tensor(e1[:], x0_sb[:, h, :], sigbB[:], ALU.mult)
            nc.vector.tensor_tensor(keepP[h][:], e1[:], pxw_T[h][:], ALU.add)

        # ---------------- main loop: stream adjt, k-combine, matmul ----------
        for mg in range(nmg):
            cs = slice(mg * mg_mc, (mg + 1) * mg_mc)
            a_k = []
            a_sb_dt = dt.float8e4 if adj_fp8_host else dt.bfloat16
            for kk in range(k_dim):
                a_t = adj_pool.tile([P, mg_mc * ns], a_sb_dt,
                                    tag=f"adj{kk}", name=f"adj_t{kk}")
                # adj stream rides the ACT HWDGE ring, separate from the
                # sync ring that carries x/consts/stores
                eng = nc.scalar if (adj_bf16_host or adj_fp8_host) else nc.gpsimd
                eng.dma_start(
                    out=a_t[:].rearrange("p (c n) -> p c n", c=mg_mc),
                    in_=adjt[kk, :, cs, :],
                )
                a_k.append(a_t)
            # comb' = a0 + (cw1/cw0)*a1  (single DVE op; GpSimd's TT and a
            # split TS+TT variant both measured slower)
            comb = comb_pool.tile([P, mg_mc * ns], dt.bfloat16, tag="comb")
            nc.vector.scalar_tensor_tensor(
                comb[:], a_k[1][:], ratio[:, 0:1], a_k[0][:], ALU.mult, ALU.add
            )
            # same-PSUM-bank runs of mg_mc matmuls (avoids bank cycling)
            for h in range(nh):
                for s in range(nw):
                    for c in range(mg_mc):
                        mc = mg * mg_mc + c
                        lhsT = x4[:, mc, 2 * h : 2 * h + 2, :].rearrange(
                            "p b f -> p (b f)"
                        )
                        nc.tensor.matmul(
                            psum_T[h][:, s * 512 : (s + 1) * 512],
                            lhsT,
                            comb[:, c * ns + s * 512 : c * ns + (s + 1) * 512],
                            start=(mc == mg * mg_mc and mg == 0),
                            stop=(mc == mc_cnt - 1 and c == mg_mc - 1),
                            skip_group_check=True,
                        )

        # ---------------- epilogue ----------------
        # out = tanh(0.5*siga*(y + cb) + xw + x0*sigb), all in [bf, n] layout
        for h in range(nh):
            e3 = work.tile([P, ns], dt.float32, tag="ew")
            nc.vector.tensor_tensor(e3[:], psum_T[h][:], siga05Bc[:], ALU.mult)
            e4 = work.tile([P, ns], dt.float32, tag="ew")
            nc.vector.tensor_tensor(e4[:], e3[:], keepP[h][:], ALU.add)
            # + 0.5*siga*conv_b
            e5 = work.tile([P, ns], dt.float32, tag="ew")
            nc.vector.scalar_tensor_tensor(
                e5[:], siga05B[:], cb_sb[:, 0:1], e4[:], ALU.mult, ALU.add
            )
            outt = outp.tile([P, ns], dt.float32, tag="outt")
            nc.scalar.activation(outt[:], e5[:], AF.Tanh)
            nc.sync.dma_start(out=y_T[h, :, :], in_=outt[:])

    nc.finalize()
    return nc


_NC_CACHE = {}


def _get_nc():
    key = (N, N_CORES, B, F, K, ADJ_BF16_HOST, ADJ_FP8_HOST)
    if key not in _NC_CACHE:
        _NC_CACHE[key] = build_kernel(
            n=N, n_cores=N_CORES, b=B, f=F, k_dim=K,
            adj_bf16_host=ADJ_BF16_HOST, adj_fp8_host=ADJ_FP8_HOST,
        )
    return _NC_CACHE[key]


def make_in_maps(x, x0, adj, alpha, beta, w, d, conv_w, conv_b, n_cores=N_CORES):
    """Host-side staging: slice rows per core and retile/transpose (pure
    layout transforms)."""
    k_dim, n, _ = adj.shape
    b, _, f = x.shape
    ns = n // n_cores
    mc_cnt = n // P
    nh = (b * f) // P
    f32 = np.float32
    adj = np.asarray(adj, dtype=f32)
    x = np.asarray(x, dtype=f32)
    x0 = np.asarray(x0, dtype=f32)
    alpha = np.asarray(alpha, dtype=f32)
    beta = np.asarray(beta, dtype=f32)

    # xres_in[p, mc, b, f] = x[b, mc*128+p, f]   (shared by all cores)
    xres_in = np.ascontiguousarray(
        x.reshape(b, mc_cnt, P, f).transpose(2, 1, 0, 3)
    )
    if ADJ_BF16_HOST or ADJ_FP8_HOST:
        import ml_dtypes

        xres_in = xres_in.astype(ml_dtypes.bfloat16)
    wT = np.ascontiguousarray(np.asarray(w, dtype=f32).T)

    in_maps = []
    for c in range(n_cores):
        rows = slice(c * ns, (c + 1) * ns)
        ac = adj[:, rows, :]  # [k, ns, n] view
        # adjt[k, p, mc, nn] = ac[k, nn, mc*128+p]
        s0, s1, s2 = ac.strides
        adjt = np.lib.stride_tricks.as_strided(
            ac, shape=(k_dim, P, mc_cnt, ns), strides=(s0, s2, P * s2, s1)
        )
        adjt = np.ascontiguousarray(adjt)
        if ADJ_FP8_HOST:
            import ml_dtypes

            adjt = (adjt * np.float32(ADJ_FP8_SCALE)).astype(
                ml_dtypes.float8_e4m3
            )
        elif ADJ_BF16_HOST:
            import ml_dtypes

            adjt = adjt.astype(ml_dtypes.bfloat16)

        xr = x[:, rows, :]  # [b, ns, f]
        xt_rows = np.ascontiguousarray(xr.transpose(2, 0, 1))  # [f, b, ns]
        # x0t_in[p, h, nn] = x0[2h + p//64, rows0+nn, p%64]
        x0t_in = np.ascontiguousarray(
            x0[:, rows, :].transpose(0, 2, 1).reshape(nh, P, ns).transpose(1, 0, 2)
        )
        ab = np.stack([alpha[rows], beta[rows]], axis=0)

        in_maps.append(
            {
                "adjt": adjt,
                "xres_in": xres_in,
                "xt_rows": xt_rows,
                "x0t_in": x0t_in,
                "ab_rows": np.ascontiguousarray(ab),
                "wT": wT,
                "d": np.ascontiguousarray(d, dtype=f32),
                "conv_w": np.ascontiguousarray(conv_w, dtype=f32),
                "conv_b": np.ascontiguousarray(conv_b, dtype=f32),
            }
        )
    return in_maps


def assemble_output(per_core_y, n_cores=N_CORES):
    """y_T[h, p, n] per core -> full [b, n, f]."""
    parts = []
    for c in range(n_cores):
        yt = per_core_y[c]  # [nh, P, ns]
        nh_, p_, ns_ = yt.shape
        b_ = nh_ * (p_ // 64)
        f_ = 64
        # [nh, P, ns] -> [b, f, ns] -> [b, ns, f]
        parts.append(
            yt.reshape(nh_, p_ // f_, f_, ns_)
            .reshape(b_, f_, ns_)
            .transpose(0, 2, 1)
        )
    return np.concatenate(parts, axis=1)


def kernel(x, x0, adj, alpha, beta, w, d, conv_w, conv_b):
    x = np.asarray(x)
    x0 = np.asarray(x0)
    adj = np.asarray(adj)
    alpha = np.asarray(alpha)
    beta = np.asarray(beta)
    w = np.asarray(w)
    d = np.asarray(d)
    conv_w = np.asarray(conv_w)
    conv_b = np.asarray(conv_b)

    nc = _get_nc()
    in_maps = make_in_maps(x, x0, adj, alpha, beta, w, d, conv_w, conv_b)
    res = run_bass_kernel_spmd(nc, in_maps, core_ids=list(range(N_CORES)))
    out = assemble_output([res.results[c]["y_T"] for c in range(N_CORES)])
    return out.astype(np.float32)


# revision 36
# speedup vs baseline: 1.0256x; 1.0210x over previous
"""Trainium2 Bass kernel for the GODEFunc graph-ODE message-passing module.

Math (per batch b):
    xa   = sum_k conv_w[k] * (adj[k] @ x[b]) + conv_b
    W'   = (w * clip(d,0,1)) @ w.T          (symmetric)
    out  = tanh(0.5*sigmoid(alpha) * xa - 2*x[b] + x[b] @ W' + x0[b]*sigmoid(beta))

Sharding: output rows (nodes) split across 8 cores; each core computes its
1024-row slice for all batches.  No collectives.

Layout decisions:
  - Host stages adj TRANSPOSED and 128-row tiled
    (adjt[k, p, mc, n] = adj[k, rows0+n, mc*128+p]) so the contraction dim m
    lands on SBUF partitions naturally -> zero on-chip transposes, and every
    DMA gets multi-KB contiguous per-partition lines.
  - adj streams as fp8e4m3 scaled by 2^12 (entries of the row-normalized
    adjacency are ~1e-4); the scale is folded back out exactly through the
    alpha gate.  x is resident as bf16; the graph matmul runs bf16 with fp32
    PSUM accumulation; the xw path and all gates are fp32.  Measured output
    error ~5e-4 vs the 2e-2 gate.
  - The matmul runs x-STATIONARY: lhsT = x4[mc] column pairs (128 rows of
    contraction x 128 (b,f) columns), moving operand = the combined adj
    stream at N=512, grouped so consecutive matmuls hit the same PSUM bank.
    The output is produced transposed ([bf, n]) and the host un-transposes.
  - Each PSUM accumulation group owns a full 2KB bank: has_written/"zero
    region" flags are PER-BANK on TRN2, so a second group's start=True in a
    shared bank silently drops the first group's accumulated partials (this
    was the old baseline's 8.7e-3 error).
  - k=0,1 combined on DVE with a single scalar_tensor_tensor per group via
    the conv_w ratio trick: comb' = a0 + (cw1/cw0)*a1, with cw0 folded into
    the alpha gate.  DVE is the pacing engine (~75us at 1 elem/lane/cycle);
    0.5*sigmoid(alpha), sigmoid(beta) apply in the epilogue via
    partition-broadcast tiles, psum-independent terms precomputed up front.
"""

import sys

for _p in ("/opt/trn_rl_repo",):
    if _p not in sys.path:
        sys.path.insert(0, _p)

from contextlib import ExitStack

import numpy as np

import concourse.bass as bass
import concourse.mybir as mybir
import concourse.tile as tile
from concourse import bacc
from concourse.bass_utils import run_bass_kernel_spmd
from concourse.masks import make_identity

dt = mybir.dt
AF = mybir.ActivationFunctionType
ALU = mybir.AluOpType

B, N, F, K = 4, 8192, 64, 2
N_CORES = 8
P = 128

# The kernel computes the adjacency matmul in bf16 either way (values are
# bit-identical); staging the bf16 cast on the host halves the HBM bytes the
# cores must stream.  The xw path and all gates stay fp32.
ADJ_BF16_HOST = True
# One step further: stage adj as fp8e4m3 scaled by 2^12 (row-normalized adj
# entries are ~1e-4; the scale is folded back out exactly via the alpha gate).
# Halves the adj stream again; the matmul itself still runs bf16 (the combine
# upconverts).  Output error ~1.5e-4 vs the 2e-2 gate.
ADJ_FP8_HOST = True
ADJ_FP8_SCALE = 4096.0


def build_kernel(n=N, n_cores=N_CORES, b=B, f=F, k_dim=K, mg_mc=2,
                 adj_bf16_host=ADJ_BF16_HOST, adj_fp8_host=ADJ_FP8_HOST):
    if adj_fp8_host:
        mg_mc = max(mg_mc, 4)  # keep adj DMAs >=512KB at 1 byte/elem
    ns = n // n_cores          # rows per core (1024)
    mc_cnt = n // P            # contraction chunks (64)
    nmg = mc_cnt // mg_mc      # adj DMA groups along contraction dim
    bf = b * f
    nh = bf // P               # output partition halves (2)
    nw = ns // 512             # 512-col slices of the row range (2)

    nc = bacc.Bacc(None, target_bir_lowering=False, debug=False)

    adj_dt = dt.bfloat16 if adj_bf16_host else dt.float32
    if adj_fp8_host:
        adj_dt = dt.float8e4
    # adjt[k, p, mc, n] = adj[k, rows0+n, mc*128+p]
    adjt = nc.dram_tensor("adjt", [k_dim, P, mc_cnt, ns], adj_dt,
                          kind="ExternalInput")
    # xres_in[p, mc, b, f] = x[b, mc*128+p, f]
    if adj_fp8_host:
        x_dt = dt.float8e4
    elif adj_bf16_host:
        x_dt = dt.bfloat16
    else:
        x_dt = dt.float32
    xres_in = nc.dram_tensor("xres_in", [P, mc_cnt, b, f], x_dt,
                             kind="ExternalInput")
    # xt_rows[f, b, n] = x[b, rows0+n, f]
    xt_rows = nc.dram_tensor("xt_rows", [f, b, ns], dt.float32,
                             kind="ExternalInput")
    # x0t_in[p, h, n] = x0[2h + p//64, rows0+n, p%64]
    x0t_in = nc.dram_tensor("x0t_in", [P, nh, ns], dt.float32,
                            kind="ExternalInput")
    # ab_bc[p, 0, n] = alpha[rows0+n]; ab_bc[p, 1, n] = beta[rows0+n]
    ab_bc = nc.dram_tensor("ab_bc", [P, 2, ns], dt.float32,
                           kind="ExternalInput")
    wT = nc.dram_tensor("wT", [f, f], dt.float32, kind="ExternalInput")
    d = nc.dram_tensor("d", [f], dt.float32, kind="ExternalInput")
    conv_w = nc.dram_tensor("conv_w", [k_dim], dt.float32, kind="ExternalInput")
    conv_b = nc.dram_tensor("conv_b", [1], dt.float32, kind="ExternalInput")
    # y_T[h, p, n] = out[2h + p//64, rows0+n, p%64]
    y_T = nc.dram_tensor("y_T", [nh, P, ns], dt.float32,
                         kind="ExternalOutput")

    with tile.TileContext(nc) as tc, ExitStack() as ctx:
        const = ctx.enter_context(tc.tile_pool(name="const", bufs=1))
        xres = ctx.enter_context(tc.tile_pool(name="xres", bufs=1))
        adj_pool = ctx.enter_context(tc.tile_pool(name="adjp", bufs=4))
        comb_pool = ctx.enter_context(tc.tile_pool(name="combp", bufs=4))
        work = ctx.enter_context(tc.tile_pool(name="work", bufs=3))
        outp = ctx.enter_context(tc.tile_pool(name="outp", bufs=2))
        psy = ctx.enter_context(tc.tile_pool(name="psy", bufs=1, space="PSUM"))
        ppxw = ctx.enter_context(tc.tile_pool(name="ppxw", bufs=1, space="PSUM"))

        # ---------------- main-stream DMAs issued first ----------------
        # resident x, all batches, contraction layout, bf16 (chunked so the
        # first matmuls can start before the whole tensor lands)
        x4 = xres.tile([P, mc_cnt, b, f],
                       dt.float8e4 if adj_fp8_host else dt.bfloat16, tag="x4")
        x4_chunk = mc_cnt // 4
        for xc in range(4):
            xs = slice(xc * x4_chunk, (xc + 1) * x4_chunk)
            if adj_bf16_host or adj_fp8_host:
                nc.sync.dma_start(out=x4[:, xs, :, :], in_=xres_in[:, xs, :, :])
            else:
                nc.gpsimd.dma_start(out=x4[:, xs, :, :], in_=xres_in[:, xs, :, :])

        # ---------------- constants / gates ----------------
        ident_f = const.tile([f, f], dt.float32, tag="ident_f")
        make_identity(nc, ident_f[:])

        wT_sb = const.tile([f, f], dt.float32, tag="wT_sb")
        nc.sync.dma_start(out=wT_sb[:], in_=wT[:, :])
        d_sb = const.tile([f, 1], dt.float32, tag="d_sb")
        nc.sync.dma_start(out=d_sb[:], in_=d[:, None])
        cw_sb = const.tile([P, k_dim], dt.float32, tag="cw_sb")
        nc.sync.dma_start(out=cw_sb[:], in_=conv_w[None, :].to_broadcast((P, k_dim)))
        cb_sb = const.tile([P, 1], dt.float32, tag="cb_sb")
        nc.sync.dma_start(out=cb_sb[:], in_=conv_b[None, :].to_broadcast((P, 1)))

        # alpha/beta broadcast down partitions, staged by the host as one
        # contiguous DMA (the to_broadcast replicate-read path is slow)
        abB = work.tile([P, 2, ns], dt.float32, tag="abB")
        nc.sync.dma_start(out=abB[:], in_=ab_bc[:, :, :])
        siga05B = const.tile([P, ns], dt.float32, tag="siga05B")
        nc.scalar.activation(siga05B[:], abB[:, 0, :], AF.Sigmoid)
        nc.vector.tensor_scalar(siga05B[:], siga05B[:], 0.5, None, ALU.mult)
        sigbB = const.tile([P, ns], dt.float32, tag="sigbB")
        nc.scalar.activation(sigbB[:], abB[:, 1, :], AF.Sigmoid)
        # conv_w ratio trick: stream comb' = a0 + (cw1/cw0)*a1 (one DVE op per
        # group), and fold the overall cw0 into the alpha gate for the psum
        # term: siga05Bc = 0.5*sigmoid(alpha)*cw0.
        ratio = const.tile([P, 1], dt.float32, tag="ratio")
        nc.vector.reciprocal(ratio[:], cw_sb[:, 0:1])
        nc.vector.tensor_scalar(ratio[:], ratio[:], cw_sb[:, 1:2], None, ALU.mult)
        siga05Bc = const.tile([P, ns], dt.float32, tag="siga05Bc")
        nc.vector.tensor_scalar(
            siga05Bc[:], siga05B[:], cw_sb[:, 0:1], None, ALU.mult
        )
        if adj_fp8_host:
            # undo the exact 2^12 host scale on the adj stream
            nc.vector.tensor_scalar(
                siga05Bc[:], siga05Bc[:], 1.0 / ADJ_FP8_SCALE, None, ALU.mult
            )

        # ---------------- xt rows (for xw), x0 ----------------
        xt_sb = const.tile([f, b, ns], dt.float32, tag="xt_sb")
        nc.sync.dma_start(out=xt_sb[:], in_=xt_rows[:, :, :])
        x0_sb = const.tile([P, nh, ns], dt.float32, tag="x0_sb")
        nc.sync.dma_start(out=x0_sb[:], in_=x0t_in[:, :, :])

        # ---------------- PSUM: 4 banks y, 4 banks pxw ----------------
        psum_T = [psy.tile([P, ns], dt.float32, tag=f"yT{h}", name=f"psum_T{h}")
                  for h in range(nh)]
        pxw_T = [ppxw.tile([P, ns], dt.float32, tag=f"pxw{h}", name=f"pxw_T{h}")
                 for h in range(nh)]

        # ---------------- W' = (w * clip(d,0,1)) @ w.T - 2I ----------------
        dc = const.tile([f, 1], dt.float32, tag="dc")
        nc.vector.tensor_scalar(dc[:], d_sb[:], 0.0, 1.0, ALU.max, ALU.min)
        wdcT = const.tile([f, f], dt.float32, tag="wdcT")
        nc.vector.tensor_scalar(wdcT[:], wT_sb[:], dc[:], None, ALU.mult)
        pw = pxw_T[0][0:f, 0:f]  # borrow a pxw bank region; consumed before pxw
        nc.tensor.matmul(pw, wdcT[:], wT_sb[:], start=True, stop=True)
        wp = const.tile([f, f], dt.float32, tag="wp")
        nc.vector.scalar_tensor_tensor(
            wp[:], ident_f[:], -2.0, pw, ALU.mult, ALU.add
        )

        # ---------------- pxw_T[h] = (x_rows @ (W'-2I)).T, transposed layout --
        # wp is symmetric, so it serves directly as lhsT.
        for h in range(nh):
            for bb in (2 * h, 2 * h + 1):
                base = (bb % 2) * f
                for s in range(nw):
                    nc.tensor.matmul(
                        pxw_T[h][base : base + f, s * 512 : (s + 1) * 512],
                        wp[:],
                        xt_sb[:, bb, s * 512 : (s + 1) * 512],
                        start=True,
                        stop=True,
                    )
        # psum-independent epilogue terms, computed up front:
        # e12[h] = x0*sigmoid(beta) + xw
        keepP = [const.tile([P, ns], dt.float32, tag=f"e12_{h}", name=f"e12_{h}")
                 for h in range(nh)]
        for h in range(nh):
            e1 = work.tile([P, ns], dt.float32, tag="ew")
            nc.vector.tensor_tensor(e1[:], x0_sb[:, h, :], sigbB[:], ALU.mult)
            nc.vector.tensor_tensor(keepP[h][:], e1[:], pxw_T[h][:], ALU.add)

        # ---------------- main loop: stream adjt, k-combine, matmul ----------
        # DMA granularity (1MB per transfer, bg = 2 combine groups) is
        # decoupled from the combine/matmul granularity (mg).
        a_sb_dt = dt.float8e4 if adj_fp8_host else dt.bfloat16
        big = {}

        def issue_adj(bg):
            cs = slice(bg * 2 * mg_mc, (bg + 1) * 2 * mg_mc)
            for kk in range(k_dim):
                a_t = adj_pool.tile([P, 2 * mg_mc * ns], a_sb_dt,
                                    tag=f"adj{kk}", name=f"adj_t{kk}")
                # adj stream rides the ACT HWDGE ring, separate from the
                # sync ring that carries x/consts/stores
                eng = nc.scalar if (adj_bf16_host or adj_fp8_host) else nc.gpsimd
                eng.dma_start(
                    out=a_t[:].rearrange("p (c n) -> p c n", c=2 * mg_mc),
                    in_=adjt[kk, :, cs, :],
                )
                big[(bg, kk)] = a_t

        nbg = nmg // 2
        for bg in range(min(2, nbg)):
            issue_adj(bg)

        for mg in range(nmg):
            bg, half = mg // 2, mg % 2
            if half == 0 and bg + 2 < nbg:
                issue_adj(bg + 2)
            hs = slice(half * mg_mc * ns, (half + 1) * mg_mc * ns)
            a_k = [big[(bg, kk)][:, hs] for kk in range(k_dim)]
            # comb' = a0 + (cw1/cw0)*a1  (single DVE op; GpSimd's TT and a
            # split TS+TT variant both measured slower)
            comb = comb_pool.tile(
                [P, mg_mc * ns],
                dt.float8e4 if adj_fp8_host else dt.bfloat16, tag="comb")
            nc.vector.scalar_tensor_tensor(
                comb[:], a_k[1], ratio[:, 0:1], a_k[0], ALU.mult, ALU.add
            )
            # same-PSUM-bank runs of mg_mc matmuls (avoids bank cycling)
            for h in range(nh):
                for s in range(nw):
                    for c in range(mg_mc):
                        mc = mg * mg_mc + c
                        lhsT = x4[:, mc, 2 * h : 2 * h + 2, :].rearrange(
                            "p b f -> p (b f)"
                        )
                        nc.tensor.matmul(
                            psum_T[h][:, s * 512 : (s + 1) * 512],
                            lhsT,
                            comb[:, c * ns + s * 512 : c * ns + (s + 1) * 512],
                            start=(mc == mg * mg_mc and mg == 0),
                            stop=(mc == mc_cnt - 1 and c == mg_mc - 1),
                            skip_group_check=True,
                        )

        # ---------------- epilogue ----------------
        # out = tanh(0.5*siga*(y + cb) + xw + x0*sigb), all in [bf, n] layout
        for h in range(nh):
            e3 = work.tile([P, ns], dt.float32, tag="ew")
            nc.vector.tensor_tensor(e3[:], psum_T[h][:], siga05Bc[:], ALU.mult)
            e4 = work.tile([P, ns], dt.float32, tag="ew")
            nc.vector.tensor_tensor(e4[:], e3[:], keepP[h][:], ALU.add)
            # + 0.5*siga*conv_b
            e5 = work.tile([P, ns], dt.float32, tag="ew")
            nc.vector.scalar_tensor_tensor(
                e5[:], siga05B[:], cb_sb[:, 0:1], e4[:], ALU.mult, ALU.add
            )
            outt = outp.tile([P, ns], dt.float32, tag="outt")
            nc.scalar.activation(outt[:], e5[:], AF.Tanh)
            nc.sync.dma_start(out=y_T[h, :, :], in_=outt[:])

    nc.finalize()
    return nc


_NC_CACHE = {}


def _get_nc():
    key = (N, N_CORES, B, F, K, ADJ_BF16_HOST, ADJ_FP8_HOST)
    if key not in _NC_CACHE:
        _NC_CACHE[key] = build_kernel(
            n=N, n_cores=N_CORES, b=B, f=F, k_dim=K,
            adj_bf16_host=ADJ_BF16_HOST, adj_fp8_host=ADJ_FP8_HOST,
        )
    return _NC_CACHE[key]


def make_in_maps(x, x0, adj, alpha, beta, w, d, conv_w, conv_b, n_cores=N_CORES):
    """Host-side staging: slice rows per core and retile/transpose (pure
    layout transforms)."""
    k_dim, n, _ = adj.shape
    b, _, f = x.shape
    ns = n // n_cores
    mc_cnt = n // P
    nh = (b * f) // P
    f32 = np.float32
    adj = np.asarray(adj, dtype=f32)
    x = np.asarray(x, dtype=f32)
    x0 = np.asarray(x0, dtype=f32)
    alpha = np.asarray(alpha, dtype=f32)
    beta = np.asarray(beta, dtype=f32)

    # xres_in[p, mc, b, f] = x[b, mc*128+p, f]   (shared by all cores)
    xres_in = np.ascontiguousarray(
        x.reshape(b, mc_cnt, P, f).transpose(2, 1, 0, 3)
    )
    if ADJ_FP8_HOST:
        import ml_dtypes

        xres_in = xres_in.astype(ml_dtypes.float8_e4m3)
    elif ADJ_BF16_HOST:
        import ml_dtypes

        xres_in = xres_in.astype(ml_dtypes.bfloat16)
    wT = np.ascontiguousarray(np.asarray(w, dtype=f32).T)

    in_maps = []
    for c in range(n_cores):
        rows = slice(c * ns, (c + 1) * ns)
        ac = adj[:, rows, :]  # [k, ns, n] view
        # adjt[k, p, mc, nn] = ac[k, nn, mc*128+p]
        s0, s1, s2 = ac.strides
        adjt = np.lib.stride_tricks.as_strided(
            ac, shape=(k_dim, P, mc_cnt, ns), strides=(s0, s2, P * s2, s1)
        )
        adjt = np.ascontiguousarray(adjt)
        if ADJ_FP8_HOST:
            import ml_dtypes

            adjt = (adjt * np.float32(ADJ_FP8_SCALE)).astype(
                ml_dtypes.float8_e4m3
            )
        elif ADJ_BF16_HOST:
            import ml_dtypes

            adjt = adjt.astype(ml_dtypes.bfloat16)

        xr = x[:, rows, :]  # [b, ns, f]
        xt_rows = np.ascontiguousarray(xr.transpose(2, 0, 1))  # [f, b, ns]
        # x0t_in[p, h, nn] = x0[2h + p//64, rows0+nn, p%64]
        x0t_in = np.ascontiguousarray(
            x0[:, rows, :].transpose(0, 2, 1).reshape(nh, P, ns).transpose(1, 0, 2)
        )
        ab = np.ascontiguousarray(
            np.broadcast_to(
                np.stack([alpha[rows], beta[rows]], axis=0)[None, :, :],
                (P, 2, ns),
            )
        )

        in_maps.append(
            {
                "adjt": adjt,
                "xres_in": xres_in,
                "xt_rows": xt_rows,
                "x0t_in": x0t_in,
                "ab_bc": ab,
                "wT": wT,
                "d": np.ascontiguousarray(d, dtype=f32),
                "conv_w": np.ascontiguousarray(conv_w, dtype=f32),
                "conv_b": np.ascontiguousarray(conv_b, dtype=f32),
            }
        )
    return in_maps


def assemble_output(per_core_y, n_cores=N_CORES):
    """y_T[h, p, n] per core -> full [b, n, f]."""
    parts = []
    for c in range(n_cores):
        yt = per_core_y[c]  # [nh, P, ns]
        nh_, p_, ns_ = yt.shape
        b_ = nh_ * (p_ // 64)
        f_ = 64
        # [nh, P, ns] -> [b, f, ns] -> [b, ns, f]
        parts.append(
            yt.reshape(nh_, p_ // f_, f_, ns_)
            .reshape(b_, f_, ns_)
            .transpose(0, 2, 1)
        )
    return np.concatenate(parts, axis=1)


def kernel(x, x0, adj, alpha, beta, w, d, conv_w, conv_b):
    x = np.asarray(x)
    x0 = np.asarray(x0)
    adj = np.asarray(adj)
    alpha = np.asarray(alpha)
    beta = np.asarray(beta)
    w = np.asarray(w)
    d = np.asarray(d)
    conv_w = np.asarray(conv_w)
    conv_b = np.asarray(conv_b)

    nc = _get_nc()
    in_maps = make_in_maps(x, x0, adj, alpha, beta, w, d, conv_w, conv_b)
    res = run_bass_kernel_spmd(nc, in_maps, core_ids=list(range(N_CORES)))
    out = assemble_output([res.results[c]["y_T"] for c in range(N_CORES)])
    return out.astype(np.float32)
